# revision 6
# baseline (speedup 1.0000x reference)
"""Contrastive loss (SimCLR-style) on 8 TRN2 NeuronCores.

loss = -mean(diag(log_softmax(zi_n @ zj_n^T / T)))  with zi_n, zj_n L2-normalized,
N=4096, D=256, T=0.5.

Strategy (data-parallel over rows, sharded preprocessing + AllGather):
  - core c gets rows [c*512, (c+1)*512) of BOTH z_i and z_j (1 MB HBM per core).
  - each core normalizes only its own z_j shard (bf16), then an AllGather of
    the scaled bf16 shards builds the full normalized z_j in DRAM; two big
    DRAM-source xbar DMA transposes produce the [d, m] operand layout.
  - rsqrt runs on VectorE (bit-trick seed + 2 Newton steps) so ScalarE's
    activation table set stays pinned to exp.
  - matmul in bf16 with f32 PSUM accumulate; fused exp+row-sum on ScalarE in
    place over PSUM (logits in [-2,2], no max subtraction needed).
  - the diagonal block of logits is the core's own z_j shard: fused
    multiply+accumulate against z_i in normal layout.
  - each core returns 4 partial sums of (lse[n] - logits[n,n]); host adds the
    32 values and divides by N.
"""

import numpy as np

import concourse.bass as bass
import concourse.bacc as bacc
import concourse.tile as tile
import concourse.bass_utils as bass_utils
from concourse import mybir

N = 4096
D = 256
NCORES = 8
NL = N // NCORES  # 512 local rows per core
P = 128
NCHUNK = NL // P  # 4 local row chunks
KH = D // P  # 2 contraction halves
MAGIC = 0x5F3759DF

F32 = mybir.dt.float32
U32 = mybir.dt.uint32
BF16 = mybir.dt.bfloat16
AF = mybir.ActivationFunctionType
ALU = mybir.AluOpType


def rsqrt_dve(nc, work, a, y, w):
    """y[:, :w] = 1/sqrt(a[:, :w]) on VectorE: quake seed + 2 Newton steps."""
    au = a.bitcast(U32)
    yu = y.bitcast(U32)
    magic = work.tile([P, w], U32, tag=f"magic{w}")
    nc.vector.memset(magic, MAGIC)
    sh = work.tile([P, w], U32, tag=f"rsq_sh{w}")
    nc.vector.tensor_scalar(
        out=sh, in0=au, scalar1=1, scalar2=None, op0=ALU.logical_shift_right
    )
    nc.vector.tensor_sub(out=yu, in0=magic, in1=sh)
    t1 = work.tile([P, w], F32, tag=f"rsq_t1{w}")
    t2 = work.tile([P, w], F32, tag=f"rsq_t2{w}")
    for _ in range(2):
        nc.vector.tensor_mul(out=t1, in0=y, in1=y)
        nc.vector.tensor_mul(out=t2, in0=t1, in1=a)
        nc.vector.tensor_scalar(
            out=t2, in0=t2, scalar1=-0.5, scalar2=1.5, op0=ALU.mult, op1=ALU.add
        )
        nc.vector.tensor_mul(out=y, in0=y, in1=t2)


def build_nc():
    nc = bacc.Bacc(
        "TRN2",
        target_bir_lowering=False,
        debug=False,
        enable_asserts=False,
        num_devices=NCORES,
    )
    z_i = nc.dram_tensor("z_i", (NL, D), F32, kind="ExternalInput").ap()
    z_jl = nc.dram_tensor("z_jl", (NL, D), F32, kind="ExternalInput").ap()
    out = nc.dram_tensor("out", (1, NCHUNK), F32, kind="ExternalOutput").ap()

    with tile.TileContext(nc) as tc:
        with (
            tc.tile_pool(name="const", bufs=1) as const,
            tc.tile_pool(name="big", bufs=1) as big,
            tc.tile_pool(name="work", bufs=4) as work,
            tc.tile_pool(name="stat", bufs=1) as stat,
            tc.tile_pool(name="dram", bufs=1, space="DRAM") as dram,
            tc.tile_pool(name="psum", bufs=2, space="PSUM") as psum,
        ):
            # --- dummy exp: force the exp ACT table set load at t=0
            dummy = const.tile([1, 1], F32)
            nc.vector.memset(dummy, 1.0)
            nc.scalar.activation(out=dummy, in_=dummy, func=AF.Exp)

            ones = const.tile([P, 1], F32)
            nc.vector.memset(ones, 1.0)

            # --- load shards, cast to bf16
            zi_bf = big.tile([P, NCHUNK, D], BF16)
            nc.gpsimd.dma_start(
                out=zi_bf, in_=z_i.rearrange("(c p) d -> p c d", p=P)
            )
            zjl_bf = big.tile([P, NCHUNK, D], BF16)
            nc.gpsimd.dma_start(
                out=zjl_bf, in_=z_jl.rearrange("(c p) d -> p c d", p=P)
            )

            # --- zj-shard norms -> t_l = 1/|zj_n|, scale rows (bf16)
            nrm2_l = stat.tile([P, NCHUNK], F32)
            for i in range(NCHUNK):
                sq = work.tile([P, D], BF16, tag="sq")
                nc.vector.scalar_tensor_tensor(
                    out=sq, in0=zjl_bf[:, i, :], scalar=1.0, in1=zjl_bf[:, i, :],
                    op0=ALU.mult, op1=ALU.mult,
                    accum_out=nrm2_l[:, i : i + 1],
                )
            t_l = stat.tile([P, NCHUNK], F32)
            rsqrt_dve(nc, work, nrm2_l, t_l, NCHUNK)
            zjls = big.tile([P, NCHUNK, D], BF16)
            for i in range(NCHUNK):
                nc.vector.tensor_scalar_mul(
                    out=zjls[:, i, :], in0=zjl_bf[:, i, :], scalar1=t_l[:, i : i + 1]
                )

            # --- AllGather the scaled bf16 shards -> full normalized z_j
            cc_in = dram.tile([NL, D], BF16)
            nc.sync.dma_start(
                out=cc_in.rearrange("(c p) d -> p c d", p=P), in_=zjls
            )
            cc_out = dram.tile([N, D], BF16)
            nc.gpsimd.collective_compute(
                "AllGather",
                ALU.bypass,
                replica_groups=[list(range(NCORES))],
                ins=[cc_in.opt()],
                outs=[cc_out.opt()],
            )

            # --- zi norms -> s2 = 2/|zi| (feeds exp scale)
            nrm2_i = stat.tile([P, NCHUNK], F32)
            for i in range(NCHUNK):
                sq = work.tile([P, D], BF16, tag="sq")
                nc.vector.scalar_tensor_tensor(
                    out=sq, in0=zi_bf[:, i, :], scalar=1.0, in1=zi_bf[:, i, :],
                    op0=ALU.mult, op1=ALU.mult,
                    accum_out=nrm2_i[:, i : i + 1],
                )
            s2 = stat.tile([P, NCHUNK], F32)
            rsqrt_dve(nc, work, nrm2_i, s2, NCHUNK)
            nc.vector.tensor_scalar(
                out=s2, in0=s2, scalar1=2.0, scalar2=None, op0=ALU.mult
            )

            # --- zi transpose via DRAM bounce: 2 big xbar transposes
            zis_dram = dram.tile([NL, D], BF16)
            nc.sync.dma_start(
                out=zis_dram.rearrange("(c p) d -> p c d", p=P), in_=zi_bf
            )
            ziT = big.tile([P, KH, NL], BF16)
            for h in range(KH):
                eng = nc.sync if h == 0 else nc.scalar
                eng.dma_start_transpose(
                    out=ziT[:, h, :], in_=zis_dram[:, h * P : (h + 1) * P]
                )

            # --- gathered z_j transposes: 4 big DRAM-source xbar transposes
            zjT = big.tile([P, KH, N], BF16)
            for h in range(KH):
                for q in range(2):
                    eng = nc.sync if (h + q) % 2 == 0 else nc.scalar
                    eng.dma_start_transpose(
                        out=zjT[:, h, q * 2048 : (q + 1) * 2048],
                        in_=cc_out[q * 2048 : (q + 1) * 2048, h * P : (h + 1) * P],
                    )

            # --- main compute: logits chunk [128, 2048] in PSUM, exp+rowsum
            # in place on ScalarE. 4 n-chunks x 2 halves.
            MW = 2048  # psum tile width
            NSL = MW // 512  # 4 matmul slices per tile
            lse_parts = stat.tile([P, 2, NCHUNK], F32)
            for i in range(NCHUNK):
                for half in range(2):
                    pt = psum.tile([P, MW], F32, tag="pt")
                    for h in range(KH):
                        for jj in range(NSL):
                            m0 = half * MW + jj * 512
                            nc.tensor.matmul(
                                pt[:, jj * 512 : (jj + 1) * 512],
                                lhsT=ziT[:, h, i * P : (i + 1) * P],
                                rhs=zjT[:, h, m0 : m0 + 512],
                                start=(h == 0),
                                stop=(h == KH - 1),
                            )
                    nc.scalar.activation(
                        out=pt,
                        in_=pt,
                        func=AF.Exp,
                        scale=s2[:, i : i + 1],
                        accum_out=lse_parts[:, half, i : i + 1],
                    )

            # --- diagonal: dt[p,i] = t_n * (zi_n . zj_n); logits diag = s2 * dt
            dt = stat.tile([P, NCHUNK], F32)
            for i in range(NCHUNK):
                sq = work.tile([P, D], BF16, tag="sq")
                nc.vector.scalar_tensor_tensor(
                    out=sq, in0=zi_bf[:, i, :], scalar=1.0, in1=zjls[:, i, :],
                    op0=ALU.mult, op1=ALU.mult,
                    accum_out=dt[:, i : i + 1],
                )
            dg = stat.tile([P, NCHUNK], F32)
            nc.vector.tensor_mul(out=dg, in0=dt, in1=s2)

            # --- lse = ln(sum of the two half row-sums); contrib = lse - diag
            rs = stat.tile([P, NCHUNK], F32)
            nc.vector.tensor_add(
                out=rs, in0=lse_parts[:, 0, :], in1=lse_parts[:, 1, :]
            )
            lse = stat.tile([P, NCHUNK], F32)
            nc.scalar.activation(out=lse, in_=rs, func=AF.Ln)
            contrib = stat.tile([P, NCHUNK], F32)
            nc.vector.tensor_sub(out=contrib, in0=lse, in1=dg)

            # --- partition reduction via ones-matmul: [1, 4] partials
            pt_fin = psum.tile([P, MW], F32, tag="pt")
            nc.tensor.matmul(
                pt_fin[:1, :NCHUNK], lhsT=ones, rhs=contrib, start=True, stop=True
            )
            osb = stat.tile([1, NCHUNK], F32)
            nc.vector.tensor_copy(out=osb, in_=pt_fin[:1, :NCHUNK])
            nc.sync.dma_start(out=out, in_=osb)

    nc.compile()
    return nc


_NC = None


def _get_nc():
    global _NC
    if _NC is None:
        _NC = build_nc()
    return _NC


def kernel(z_i: np.ndarray, z_j: np.ndarray, **_unused) -> np.ndarray:
    z_i = np.ascontiguousarray(z_i, dtype=np.float32)
    z_j = np.ascontiguousarray(z_j, dtype=np.float32)
    nc = _get_nc()
    in_maps = []
    for c in range(NCORES):
        sl = slice(c * NL, (c + 1) * NL)
        in_maps.append(
            {
                "z_i": z_i[sl],
                "z_jl": z_j[sl],
            }
        )
    res = bass_utils.run_bass_kernel_spmd(
        nc, in_maps, core_ids=list(range(NCORES))
    )
    total = 0.0
    for c in range(NCORES):
        total += float(res.results[c]["out"].astype(np.float64).sum())
    return np.float32(total / N)


# revision 7
# speedup vs baseline: 1.1000x; 1.1000x over previous
"""Contrastive loss (SimCLR-style) on 8 TRN2 NeuronCores.

loss = -mean(diag(log_softmax(zi_n @ zj_n^T / T)))  with zi_n, zj_n L2-normalized,
N=4096, D=256, T=0.5.

Strategy (data-parallel over rows, sharded preprocessing + AllGather):
  - core c gets rows [c*512, (c+1)*512) of BOTH z_i and z_j (1 MB HBM per core).
  - each core normalizes only its own z_j shard (bf16), then an AllGather of
    the scaled bf16 shards builds the full normalized z_j in DRAM; two big
    DRAM-source xbar DMA transposes produce the [d, m] operand layout.
  - rsqrt runs on VectorE (bit-trick seed + 2 Newton steps) so ScalarE's
    activation table set stays pinned to exp.
  - matmul in bf16 with f32 PSUM accumulate; fused exp+row-sum on ScalarE in
    place over PSUM (logits in [-2,2], no max subtraction needed).
  - the diagonal block of logits is the core's own z_j shard: fused
    multiply+accumulate against z_i in normal layout.
  - each core returns 4 partial sums of (lse[n] - logits[n,n]); host adds the
    32 values and divides by N.
"""

import numpy as np

import concourse.bass as bass
import concourse.bacc as bacc
import concourse.tile as tile
import concourse.bass_utils as bass_utils
from concourse import mybir

N = 4096
D = 256
NCORES = 8
NL = N // NCORES  # 512 local rows per core
P = 128
NCHUNK = NL // P  # 4 local row chunks
KH = D // P  # 2 contraction halves
MAGIC = 0x5F3759DF

F32 = mybir.dt.float32
U32 = mybir.dt.uint32
BF16 = mybir.dt.bfloat16
AF = mybir.ActivationFunctionType
ALU = mybir.AluOpType


def rsqrt_dve(nc, work, a, y, w):
    """y[:, :w] = 1/sqrt(a[:, :w]) on VectorE: quake seed + 2 Newton steps."""
    au = a.bitcast(U32)
    yu = y.bitcast(U32)
    magic = work.tile([P, w], U32, tag=f"magic{w}")
    nc.vector.memset(magic, MAGIC)
    sh = work.tile([P, w], U32, tag=f"rsq_sh{w}")
    nc.vector.tensor_scalar(
        out=sh, in0=au, scalar1=1, scalar2=None, op0=ALU.logical_shift_right
    )
    nc.vector.tensor_sub(out=yu, in0=magic, in1=sh)
    t1 = work.tile([P, w], F32, tag=f"rsq_t1{w}")
    t2 = work.tile([P, w], F32, tag=f"rsq_t2{w}")
    for _ in range(2):
        nc.vector.tensor_mul(out=t1, in0=y, in1=y)
        nc.vector.tensor_mul(out=t2, in0=t1, in1=a)
        nc.vector.tensor_scalar(
            out=t2, in0=t2, scalar1=-0.5, scalar2=1.5, op0=ALU.mult, op1=ALU.add
        )
        nc.vector.tensor_mul(out=y, in0=y, in1=t2)


def build_nc():
    nc = bacc.Bacc(
        "TRN2",
        target_bir_lowering=False,
        debug=False,
        enable_asserts=False,
        num_devices=NCORES,
    )
    z_i = nc.dram_tensor("z_i", (NL, D), F32, kind="ExternalInput").ap()
    z_jl = nc.dram_tensor("z_jl", (NL, D), F32, kind="ExternalInput").ap()
    out = nc.dram_tensor("out", (1, NCHUNK), F32, kind="ExternalOutput").ap()

    with tile.TileContext(nc) as tc:
        with (
            tc.tile_pool(name="const", bufs=1) as const,
            tc.tile_pool(name="big", bufs=1) as big,
            tc.tile_pool(name="work", bufs=4) as work,
            tc.tile_pool(name="stat", bufs=1) as stat,
            tc.tile_pool(name="dram", bufs=1, space="DRAM") as dram,
            tc.tile_pool(name="psum", bufs=2, space="PSUM") as psum,
        ):
            # --- dummy exp: force the exp ACT table set load at t=0
            dummy = const.tile([1, 1], F32)
            nc.vector.memset(dummy, 1.0)
            nc.scalar.activation(out=dummy, in_=dummy, func=AF.Exp)

            ones = const.tile([P, 1], F32)
            nc.vector.memset(ones, 1.0)

            # --- load shards, cast to bf16
            zi_bf = big.tile([P, NCHUNK, D], BF16)
            nc.gpsimd.dma_start(
                out=zi_bf, in_=z_i.rearrange("(c p) d -> p c d", p=P)
            )
            zjl_bf = big.tile([P, NCHUNK, D], BF16)
            nc.gpsimd.dma_start(
                out=zjl_bf, in_=z_jl.rearrange("(c p) d -> p c d", p=P)
            )

            # --- zj-shard norms -> t_l = 1/|zj_n|, scale rows (bf16)
            nrm2_l = stat.tile([P, NCHUNK], F32)
            for i in range(NCHUNK):
                sq = work.tile([P, D], BF16, tag="sq")
                nc.vector.scalar_tensor_tensor(
                    out=sq, in0=zjl_bf[:, i, :], scalar=1.0, in1=zjl_bf[:, i, :],
                    op0=ALU.mult, op1=ALU.mult,
                    accum_out=nrm2_l[:, i : i + 1],
                )
            t_l = stat.tile([P, NCHUNK], F32)
            rsqrt_dve(nc, work, nrm2_l, t_l, NCHUNK)
            zjls = big.tile([P, NCHUNK, D], BF16)
            for i in range(NCHUNK):
                nc.vector.tensor_scalar_mul(
                    out=zjls[:, i, :], in0=zjl_bf[:, i, :], scalar1=t_l[:, i : i + 1]
                )

            # --- AllGather the scaled bf16 shards -> full normalized z_j
            cc_in = dram.tile([NL, D], BF16)
            nc.sync.dma_start(
                out=cc_in.rearrange("(c p) d -> p c d", p=P), in_=zjls
            )
            cc_out = dram.tile([N, D], BF16, addr_space="Shared")
            nc.gpsimd.collective_compute(
                "AllGather",
                ALU.bypass,
                replica_groups=[list(range(NCORES))],
                ins=[cc_in.opt()],
                outs=[cc_out.opt()],
            )

            # --- zi norms -> s2 = 2/|zi| (feeds exp scale)
            nrm2_i = stat.tile([P, NCHUNK], F32)
            for i in range(NCHUNK):
                sq = work.tile([P, D], BF16, tag="sq")
                nc.vector.scalar_tensor_tensor(
                    out=sq, in0=zi_bf[:, i, :], scalar=1.0, in1=zi_bf[:, i, :],
                    op0=ALU.mult, op1=ALU.mult,
                    accum_out=nrm2_i[:, i : i + 1],
                )
            s2 = stat.tile([P, NCHUNK], F32)
            rsqrt_dve(nc, work, nrm2_i, s2, NCHUNK)
            nc.vector.tensor_scalar(
                out=s2, in0=s2, scalar1=2.0, scalar2=None, op0=ALU.mult
            )

            # --- zi transpose via DRAM bounce: 2 big xbar transposes
            zis_dram = dram.tile([NL, D], BF16)
            nc.sync.dma_start(
                out=zis_dram.rearrange("(c p) d -> p c d", p=P), in_=zi_bf
            )
            ziT = big.tile([P, KH, NL], BF16)
            for h in range(KH):
                eng = nc.sync if h == 0 else nc.scalar
                eng.dma_start_transpose(
                    out=ziT[:, h, :], in_=zis_dram[:, h * P : (h + 1) * P]
                )

            # --- gathered z_j transposes: 4 big DRAM-source xbar transposes
            zjT = big.tile([P, KH, N], BF16)
            for h in range(KH):
                for q in range(2):
                    eng = nc.sync if (h + q) % 2 == 0 else nc.scalar
                    eng.dma_start_transpose(
                        out=zjT[:, h, q * 2048 : (q + 1) * 2048],
                        in_=cc_out[q * 2048 : (q + 1) * 2048, h * P : (h + 1) * P],
                    )

            # --- main compute: logits chunk [128, 2048] in PSUM, exp+rowsum
            # in place on ScalarE. 4 n-chunks x 2 halves.
            MW = 2048  # psum tile width
            NSL = MW // 512  # 4 matmul slices per tile
            lse_parts = stat.tile([P, 2, NCHUNK], F32)
            for i in range(NCHUNK):
                for half in range(2):
                    pt = psum.tile([P, MW], F32, tag="pt")
                    for h in range(KH):
                        for jj in range(NSL):
                            m0 = half * MW + jj * 512
                            nc.tensor.matmul(
                                pt[:, jj * 512 : (jj + 1) * 512],
                                lhsT=ziT[:, h, i * P : (i + 1) * P],
                                rhs=zjT[:, h, m0 : m0 + 512],
                                start=(h == 0),
                                stop=(h == KH - 1),
                            )
                    nc.scalar.activation(
                        out=pt,
                        in_=pt,
                        func=AF.Exp,
                        scale=s2[:, i : i + 1],
                        accum_out=lse_parts[:, half, i : i + 1],
                    )

            # --- diagonal: dt[p,i] = t_n * (zi_n . zj_n); logits diag = s2 * dt
            dt = stat.tile([P, NCHUNK], F32)
            for i in range(NCHUNK):
                sq = work.tile([P, D], BF16, tag="sq")
                nc.vector.scalar_tensor_tensor(
                    out=sq, in0=zi_bf[:, i, :], scalar=1.0, in1=zjls[:, i, :],
                    op0=ALU.mult, op1=ALU.mult,
                    accum_out=dt[:, i : i + 1],
                )
            dg = stat.tile([P, NCHUNK], F32)
            nc.vector.tensor_mul(out=dg, in0=dt, in1=s2)

            # --- lse = ln(sum of the two half row-sums); contrib = lse - diag
            rs = stat.tile([P, NCHUNK], F32)
            nc.vector.tensor_add(
                out=rs, in0=lse_parts[:, 0, :], in1=lse_parts[:, 1, :]
            )
            lse = stat.tile([P, NCHUNK], F32)
            nc.scalar.activation(out=lse, in_=rs, func=AF.Ln)
            contrib = stat.tile([P, NCHUNK], F32)
            nc.vector.tensor_sub(out=contrib, in0=lse, in1=dg)

            # --- partition reduction via ones-matmul: [1, 4] partials
            pt_fin = psum.tile([P, MW], F32, tag="pt")
            nc.tensor.matmul(
                pt_fin[:1, :NCHUNK], lhsT=ones, rhs=contrib, start=True, stop=True
            )
            osb = stat.tile([1, NCHUNK], F32)
            nc.vector.tensor_copy(out=osb, in_=pt_fin[:1, :NCHUNK])
            nc.sync.dma_start(out=out, in_=osb)

    nc.compile()
    return nc


_NC = None


def _get_nc():
    global _NC
    if _NC is None:
        _NC = build_nc()
    return _NC


def kernel(z_i: np.ndarray, z_j: np.ndarray, **_unused) -> np.ndarray:
    z_i = np.ascontiguousarray(z_i, dtype=np.float32)
    z_j = np.ascontiguousarray(z_j, dtype=np.float32)
    nc = _get_nc()
    in_maps = []
    for c in range(NCORES):
        sl = slice(c * NL, (c + 1) * NL)
        in_maps.append(
            {
                "z_i": z_i[sl],
                "z_jl": z_j[sl],
            }
        )
    res = bass_utils.run_bass_kernel_spmd(
        nc, in_maps, core_ids=list(range(NCORES))
    )
    total = 0.0
    for c in range(NCORES):
        total += float(res.results[c]["out"].astype(np.float64).sum())
    return np.float32(total / N)


# revision 8
# speedup vs baseline: 1.8615x; 1.6923x over previous
"""Contrastive loss (SimCLR-style) on 8 TRN2 NeuronCores.

loss = -mean(diag(log_softmax(zi_n @ zj_n^T / T)))  with zi_n, zj_n L2-normalized,
N=4096, D=256, T=0.5.

Strategy (data-parallel over rows of z_i, z_j replicated):
  - core c gets rows [c*512, (c+1)*512) of z_i, the full z_j, and z_j's
    matching diagonal block as a separate small input.
  - cast to bf16 during load; row norms via fused multiply+accumulate
    (scalar_tensor_tensor); rsqrt on VectorE (bit-trick + 2 Newton steps) so
    ScalarE's table set stays pinned to exp; scale z_j rows; bounce scaled
    bf16 through DRAM for big DRAM-source xbar DMA transposes; matmul in bf16
    with f32 PSUM accumulate; fused exp+row-sum on ScalarE in place over PSUM
    (logits in [-2,2]: no max subtraction); diagonal via fused
    multiply+accumulate in normal layout; ones-matmul partition reduction.
  - z_j is processed in 4 pipelined groups; the logits loop runs
    half-m-range-outer so compute on groups 0-1 overlaps preprocessing of
    groups 2-3.
  - each core returns 4 partial sums of (lse[n] - logits[n,n]); host adds the
    32 values and divides by N.
"""

import numpy as np

import concourse.bass as bass
import concourse.bacc as bacc
import concourse.tile as tile
import concourse.bass_utils as bass_utils
from concourse import mybir

N = 4096
D = 256
NCORES = 8
NL = N // NCORES  # 512 local rows per core
P = 128
NCHUNK = NL // P  # 4 local row chunks
MCHUNK = N // P  # 32 zj chunks
NGROUP = 4  # zj processed in 4 groups of 8 chunks
GCH = MCHUNK // NGROUP  # 8 chunks per group
GM = GCH * P  # 1024 rows per group
KH = D // P  # 2 contraction halves
MAGIC = 0x5F3759DF

F32 = mybir.dt.float32
U32 = mybir.dt.uint32
BF16 = mybir.dt.bfloat16
AF = mybir.ActivationFunctionType
ALU = mybir.AluOpType


def rsqrt_dve(nc, work, a, y, w):
    """y[:, :w] = 1/sqrt(a[:, :w]) on VectorE: quake seed + 2 Newton steps."""
    au = a.bitcast(U32)
    yu = y.bitcast(U32)
    magic = work.tile([P, w], U32, tag=f"magic{w}")
    nc.vector.memset(magic, MAGIC)
    sh = work.tile([P, w], U32, tag=f"rsq_sh{w}")
    nc.vector.tensor_scalar(
        out=sh, in0=au, scalar1=1, scalar2=None, op0=ALU.logical_shift_right
    )
    nc.vector.tensor_sub(out=yu, in0=magic, in1=sh)
    t1 = work.tile([P, w], F32, tag=f"rsq_t1{w}")
    t2 = work.tile([P, w], F32, tag=f"rsq_t2{w}")
    for _ in range(2):
        nc.vector.tensor_mul(out=t1, in0=y, in1=y)
        nc.vector.tensor_mul(out=t2, in0=t1, in1=a)
        nc.vector.tensor_scalar(
            out=t2, in0=t2, scalar1=-0.5, scalar2=1.5, op0=ALU.mult, op1=ALU.add
        )
        nc.vector.tensor_mul(out=y, in0=y, in1=t2)


def build_nc():
    nc = bacc.Bacc(
        "TRN2",
        target_bir_lowering=False,
        debug=False,
        enable_asserts=False,
    )
    z_i = nc.dram_tensor("z_i", (NL, D), F32, kind="ExternalInput").ap()
    z_j = nc.dram_tensor("z_j", (N, D), F32, kind="ExternalInput").ap()
    z_jd = nc.dram_tensor("z_jd", (NL, D), F32, kind="ExternalInput").ap()
    out = nc.dram_tensor("out", (1, NCHUNK), F32, kind="ExternalOutput").ap()

    with tile.TileContext(nc) as tc:
        with (
            tc.tile_pool(name="const", bufs=1) as const,
            tc.tile_pool(name="big", bufs=1) as big,
            tc.tile_pool(name="work", bufs=4) as work,
            tc.tile_pool(name="stat", bufs=1) as stat,
            tc.tile_pool(name="dram", bufs=1, space="DRAM") as dram,
            tc.tile_pool(name="psum", bufs=2, space="PSUM") as psum,
        ):
            # --- dummy exp: force the exp ACT table set load at t=0
            dummy = const.tile([1, 1], F32)
            nc.vector.memset(dummy, 1.0)
            nc.scalar.activation(out=dummy, in_=dummy, func=AF.Exp)

            ones = const.tile([P, 1], F32)
            nc.vector.memset(ones, 1.0)

            # --- small inputs: zi shard and zj diagonal block, cast to bf16
            zi_bf = big.tile([P, NCHUNK, D], BF16)
            nc.gpsimd.dma_start(
                out=zi_bf, in_=z_i.rearrange("(c p) d -> p c d", p=P)
            )
            zjd_bf = big.tile([P, NCHUNK, D], BF16)
            nc.gpsimd.dma_start(
                out=zjd_bf, in_=z_jd.rearrange("(c p) d -> p c d", p=P)
            )

            # --- zi norms -> s2 = 2/|zi| ; zi transpose via DRAM bounce
            nrm2_i = stat.tile([P, NCHUNK], F32)
            for i in range(NCHUNK):
                sq = work.tile([P, D], BF16, tag="sq")
                nc.vector.scalar_tensor_tensor(
                    out=sq, in0=zi_bf[:, i, :], scalar=1.0, in1=zi_bf[:, i, :],
                    op0=ALU.mult, op1=ALU.mult,
                    accum_out=nrm2_i[:, i : i + 1],
                )
            s2 = stat.tile([P, NCHUNK], F32)
            rsqrt_dve(nc, work, nrm2_i, s2, NCHUNK)
            nc.vector.tensor_scalar(
                out=s2, in0=s2, scalar1=2.0, scalar2=None, op0=ALU.mult
            )
            zis_dram = dram.tile([NL, D], BF16)
            nc.sync.dma_start(
                out=zis_dram.rearrange("(c p) d -> p c d", p=P), in_=zi_bf
            )
            ziT = big.tile([P, KH, NL], BF16)
            for h in range(KH):
                eng = nc.sync if h == 0 else nc.scalar
                eng.dma_start_transpose(
                    out=ziT[:, h, :], in_=zis_dram[:, h * P : (h + 1) * P]
                )

            # --- per-group zj preprocessing: load -> norms -> rsqrt -> scale
            #     -> DRAM bounce -> 2 big xbar transposes
            nrm2_j = stat.tile([P, MCHUNK], F32)
            t_j = stat.tile([P, MCHUNK], F32)
            zjT_g = []

            def zj_group(g):
                zj_bf = big.tile([P, GCH, D], BF16, tag=f"zjbf{g}")
                nc.gpsimd.dma_start(
                    out=zj_bf,
                    in_=z_j[g * GM : (g + 1) * GM, :].rearrange(
                        "(c p) d -> p c d", p=P
                    ),
                )
                for jl in range(GCH):
                    j = g * GCH + jl
                    sq = work.tile([P, D], BF16, tag="sq")
                    nc.vector.scalar_tensor_tensor(
                        out=sq, in0=zj_bf[:, jl, :], scalar=1.0,
                        in1=zj_bf[:, jl, :],
                        op0=ALU.mult, op1=ALU.mult,
                        accum_out=nrm2_j[:, j : j + 1],
                    )
                gs = slice(g * GCH, (g + 1) * GCH)
                rsqrt_dve(nc, work, nrm2_j[:, gs], t_j[:, gs], GCH)
                zjs = big.tile([P, GCH, D], BF16, tag=f"zjs{g}")
                for jl in range(GCH):
                    j = g * GCH + jl
                    nc.vector.tensor_scalar_mul(
                        out=zjs[:, jl, :],
                        in0=zj_bf[:, jl, :],
                        scalar1=t_j[:, j : j + 1],
                    )
                zjs_dram = dram.tile([GM, D], BF16, tag=f"zjsd{g}")
                nc.sync.dma_start(
                    out=zjs_dram.rearrange("(c p) d -> p c d", p=P), in_=zjs
                )
                zjT = big.tile([P, KH, GM], BF16, tag=f"zjT{g}")
                zjT_g.append(zjT)
                for h in range(KH):
                    eng = nc.sync if h == 0 else nc.scalar
                    eng.dma_start_transpose(
                        out=zjT[:, h, :], in_=zjs_dram[:, h * P : (h + 1) * P]
                    )

            # --- main compute helper: one [128, 2048] logits tile + fused exp
            MW = 2048
            NSL = MW // 512
            lse_parts = stat.tile([P, 2, NCHUNK], F32)

            def logits_tile(i, half):
                pt = psum.tile([P, MW], F32, tag="pt")
                for h in range(KH):
                    for jj in range(NSL):
                        m0 = half * MW + jj * 512
                        g = m0 // GM
                        mo = m0 % GM
                        nc.tensor.matmul(
                            pt[:, jj * 512 : (jj + 1) * 512],
                            lhsT=ziT[:, h, i * P : (i + 1) * P],
                            rhs=zjT_g[g][:, h, mo : mo + 512],
                            start=(h == 0),
                            stop=(h == KH - 1),
                        )
                nc.scalar.activation(
                    out=pt,
                    in_=pt,
                    func=AF.Exp,
                    scale=s2[:, i : i + 1],
                    accum_out=lse_parts[:, half, i : i + 1],
                )

            # pipeline: groups 0-1, then half-0 tiles while groups 2-3 preprocess
            zj_group(0)
            zj_group(1)
            for i in range(NCHUNK):
                logits_tile(i, 0)
            zj_group(2)
            zj_group(3)
            for i in range(NCHUNK):
                logits_tile(i, 1)

            # --- diagonal: t_d from zjd norms; dt = t_n*(zi.zj_n); diag = s2*dt
            nrm2_d = stat.tile([P, NCHUNK], F32)
            for i in range(NCHUNK):
                sq = work.tile([P, D], BF16, tag="sq")
                nc.vector.scalar_tensor_tensor(
                    out=sq, in0=zjd_bf[:, i, :], scalar=1.0, in1=zjd_bf[:, i, :],
                    op0=ALU.mult, op1=ALU.mult,
                    accum_out=nrm2_d[:, i : i + 1],
                )
            t_d = stat.tile([P, NCHUNK], F32)
            rsqrt_dve(nc, work, nrm2_d, t_d, NCHUNK)
            zjds = big.tile([P, NCHUNK, D], BF16)
            for i in range(NCHUNK):
                nc.vector.tensor_scalar_mul(
                    out=zjds[:, i, :], in0=zjd_bf[:, i, :], scalar1=t_d[:, i : i + 1]
                )
            dt = stat.tile([P, NCHUNK], F32)
            for i in range(NCHUNK):
                sq = work.tile([P, D], BF16, tag="sq")
                nc.vector.scalar_tensor_tensor(
                    out=sq, in0=zi_bf[:, i, :], scalar=1.0, in1=zjds[:, i, :],
                    op0=ALU.mult, op1=ALU.mult,
                    accum_out=dt[:, i : i + 1],
                )
            dg = stat.tile([P, NCHUNK], F32)
            nc.vector.tensor_mul(out=dg, in0=dt, in1=s2)

            # --- lse = ln(sum of the two half row-sums); contrib = lse - diag
            rs = stat.tile([P, NCHUNK], F32)
            nc.vector.tensor_add(
                out=rs, in0=lse_parts[:, 0, :], in1=lse_parts[:, 1, :]
            )
            lse = stat.tile([P, NCHUNK], F32)
            nc.scalar.activation(out=lse, in_=rs, func=AF.Ln)
            contrib = stat.tile([P, NCHUNK], F32)
            nc.vector.tensor_sub(out=contrib, in0=lse, in1=dg)

            # --- partition reduction via ones-matmul: [1, 4] partials
            pt_fin = psum.tile([P, MW], F32, tag="pt")
            nc.tensor.matmul(
                pt_fin[:1, :NCHUNK], lhsT=ones, rhs=contrib, start=True, stop=True
            )
            osb = stat.tile([1, NCHUNK], F32)
            nc.vector.tensor_copy(out=osb, in_=pt_fin[:1, :NCHUNK])
            nc.sync.dma_start(out=out, in_=osb)

    nc.compile()
    return nc


_NC = None


def _get_nc():
    global _NC
    if _NC is None:
        _NC = build_nc()
    return _NC


def kernel(z_i: np.ndarray, z_j: np.ndarray, **_unused) -> np.ndarray:
    z_i = np.ascontiguousarray(z_i, dtype=np.float32)
    z_j = np.ascontiguousarray(z_j, dtype=np.float32)
    nc = _get_nc()
    in_maps = []
    for c in range(NCORES):
        sl = slice(c * NL, (c + 1) * NL)
        in_maps.append(
            {
                "z_i": z_i[sl],
                "z_j": z_j,
                "z_jd": z_j[sl],
            }
        )
    res = bass_utils.run_bass_kernel_spmd(
        nc, in_maps, core_ids=list(range(NCORES))
    )
    total = 0.0
    for c in range(NCORES):
        total += float(res.results[c]["out"].astype(np.float64).sum())
    return np.float32(total / N)
